# revision 2
# baseline (speedup 1.0000x reference)
"""Causal self-attention kernel for Trainium2, data-parallel over batch on 8 cores.

v5: software-pipelined, engine-balanced rewrite.

Reference computation (B=256, T=256, C=192, H=6, D=32):
    qkv = x @ w_qkv.T -> q,k,v ; scores = q k^T / sqrt(D) causal-masked
    y = softmax(scores) @ v ; out = y @ w_out.T

Per-core design (32 batches/core, fp16 matmul operands, fp32 accumulation):
  - causal mask accumulated on the PE (triangular -3e4 stationary @ identity)
    into the score psum before a single exp on ScalarE (masked lanes
    underflow to 0) -- no gpsimd affine_select pass.
  - per-iteration interleaved software pipeline over batches (i, j=i-1,
    k=i-2) so each engine's queue stays dense:
      trans(i) | attn_a(j) | load(i+1) | attn_b(j) | qkv(i) | attn_c(j)
      | outproj(k)
  - psum bank budget (8 banks): score pairs 2x2 ('sc', ring2) + sums 1
    + av 1 + 'fr' ring2 (xt, qk01, qk2, v, o) 2. 'fr' ring waits are all
    copy-gated (never exp-gated), so front/out work never serializes
    behind the softmax chain.
"""
import sys

sys.path.insert(0, "/opt/trn_rl_repo")

import numpy as np

B, T, C, H, D = 256, 256, 192, 6, 32
NCORES = 8
BPC = B // NCORES  # 32 batches per core
SCALE = 1.0 / np.sqrt(np.float32(D))
MASKVAL = -30000.0

# column offsets of each head's q^T / k^T block inside the [32, 3072] shuffle
QCOL = [0, 768, 1536, 2304, 256, 1024]
KCOL = [1792, 2560, 512, 1280, 2048, 2816]

_CACHE = {}


def _build(bpc=BPC, repeat=0):
    import contextlib
    from concourse import bacc, tile, mybir
    from concourse.masks import make_identity, make_upper_triangular

    F32 = mybir.dt.float32
    F16 = mybir.dt.float16
    Exp = mybir.ActivationFunctionType.Exp

    nc = bacc.Bacc(None, target_bir_lowering=False)
    x_d = nc.dram_tensor("x", [bpc, T, C], F32, kind="ExternalInput")
    wqkv_d = nc.dram_tensor("w_qkv", [3 * C, C], F32, kind="ExternalInput")
    wout_d = nc.dram_tensor("w_out", [C, C], F32, kind="ExternalInput")
    out_d = nc.dram_tensor("out", [bpc, T, C], F32, kind="ExternalOutput")

    with tile.TileContext(nc) as tc:
        with tc.tile_pool(name="cst", bufs=1) as cst, \
             tc.tile_pool(name="sb", bufs=2) as sb, \
             tc.tile_pool(name="ps", bufs=1, space="PSUM") as ps:
            ident = cst.tile([128, 128], F32)
            make_identity(nc, ident[:])
            ident16 = cst.tile([128, 128], F16)
            nc.vector.tensor_copy(ident16[:], ident[:])
            ones16 = cst.tile([128, 32], F16)
            nc.vector.memset(ones16[:], 1.0)
            # strict-upper triangular MASKVAL; as matmul stationary against
            # identity it adds MASKVAL below the diagonal of S^T[tk,tq]
            tri = cst.tile([128, 128], F32)
            make_upper_triangular(nc, tri[:], val=MASKVAL, diag=False)
            tri16 = cst.tile([128, 128], F16)
            nc.vector.tensor_copy(tri16[:], tri[:])

            # ---- one-time: transpose w_qkv -> wqT fp16 [2][96, 576] ----
            wq_sb = cst.tile([128, 5, 192], F32)
            wq_v = wqkv_d[0:512, :].rearrange("(n p) c -> p n c", p=128)
            nc.sync.dma_start(wq_sb[:, 0:4, :], wq_v)
            nc.sync.dma_start(wq_sb[0:64, 4, :], wqkv_d[512:576, :])
            wqT = []
            for cb in range(2):
                w16 = cst.tile([96, 576], F16, name=f"wqT{cb}")
                wt = ps.tile([96, 512], F32, tag="fr", name=f"wtA{cb}")
                for ot in range(4):
                    nc.tensor.transpose(
                        wt[:, ot * 128:ot * 128 + 128],
                        wq_sb[0:128, ot, cb * 96:cb * 96 + 96],
                        ident[0:128, 0:128])
                nc.vector.tensor_copy(w16[:, 0:512], wt[:])
                wt2 = ps.tile([96, 512], F32, tag="fr", name=f"wtB{cb}")
                nc.tensor.transpose(
                    wt2[:, 0:64],
                    wq_sb[0:64, 4, cb * 96:cb * 96 + 96],
                    ident[0:64, 0:64])
                nc.vector.tensor_copy(w16[:, 512:576], wt2[:, 0:64])
                wqT.append(w16)

            # ---- one-time: transpose w_out -> woT fp16 [128,192]+[64,192] ----
            wo_sb = cst.tile([128, 2, 192], F32)
            nc.sync.dma_start(wo_sb[:, 0, :], wout_d[0:128, :])
            nc.sync.dma_start(wo_sb[0:64, 1, :], wout_d[128:192, :])
            woT = []
            for cb, (p0, rows) in enumerate([(0, 128), (128, 64)]):
                wt_ps = ps.tile([rows, 256], F32, tag="fr", name=f"wo_ps{cb}")
                for ot, (q0, cols) in enumerate([(0, 128), (128, 64)]):
                    nc.tensor.transpose(
                        wt_ps[:, ot * 128:ot * 128 + cols],
                        wo_sb[0:cols, ot, p0:p0 + rows],
                        ident[0:cols, 0:cols])
                w16 = cst.tile([rows, 192], F16, name=f"woT{cb}")
                nc.vector.tensor_copy(w16[:], wt_ps[:, 0:192])
                woT.append(w16)

            x_v = x_d.rearrange("b (u p) c -> b p u c", p=128)
            o_v = out_d.rearrange("b (u p) c -> b p u c", p=128)

            state = {}

            def load(i):
                x_sb = sb.tile([128, 2, 192], F32, tag="x", name=f"x{i}")
                nc.sync.dma_start(x_sb[:], x_v[i])
                x16 = sb.tile([128, 2, 192], F16, tag="x16", name=f"x16_{i}")
                nc.gpsimd.tensor_copy(x16[:], x_sb[:])
                state[i] = {"x16": x16}

            def trans(i):
                st = state[i]
                x16 = st.pop("x16")
                xt_ps = ps.tile([96, 512], F16, tag="fr", name=f"xt{i}")
                for u in range(2):
                    for cb in range(2):
                        nc.tensor.transpose(
                            xt_ps[:, cb * 256 + u * 128:cb * 256 + u * 128 + 128],
                            x16[:, u, cb * 96:cb * 96 + 96], ident16[:])
                xt16 = sb.tile([96, 512], F16, tag="xt16", name=f"xt16_{i}")
                nc.scalar.copy(xt16[:], xt_ps[:])
                st["xt16"] = xt16

            def front_qkv(i):
                st = state[i]
                xt16 = st.pop("xt16")
                qk_ps01 = ps.tile([128, 512], F32, tag="fr", name=f"qk01_{i}")
                qk_ps2 = ps.tile([128, 256], F32, tag="fr", name=f"qk2_{i}")
                for ot in range(3):
                    dst = qk_ps01[:, ot * 256:ot * 256 + 256] if ot < 2 \
                        else qk_ps2[:]
                    for cb in range(2):
                        nc.tensor.matmul(
                            dst,
                            wqT[cb][:, ot * 128:ot * 128 + 128],
                            xt16[:, cb * 256:cb * 256 + 256],
                            start=(cb == 0), stop=(cb == 1))
                qk16 = sb.tile([128, 768], F16, tag="qk16", name=f"qk16_{i}")
                nc.vector.tensor_copy(qk16[:, 0:512], qk_ps01[:])
                nc.vector.tensor_copy(qk16[:, 512:768], qk_ps2[:])
                # DMA partition-quadrant shuffle -> [32, 3072]
                qkT32 = sb.tile([32, 3072], F16, tag="qkT32", name=f"qkT32_{i}")
                for g in range(4):
                    nc.sync.dma_start(qkT32[0:32, g * 768:(g + 1) * 768],
                                      qk16[32 * g:32 * g + 32, :])
                # v in [t, o] layout
                v_ps = ps.tile([128, 384], F32, tag="fr", name=f"v{i}")
                for u in range(2):
                    for cb in range(2):
                        nc.tensor.matmul(
                            v_ps[:, u * 192:u * 192 + 192],
                            xt16[:, cb * 256 + u * 128:cb * 256 + u * 128 + 128],
                            wqT[cb][:, 384:576],
                            start=(cb == 0), stop=(cb == 1))
                v16 = sb.tile([128, 2, 192], F16, tag="v16", name=f"v16_{i}")
                nc.vector.tensor_copy(v16[:], v_ps[:])
                st["qkT32"] = qkT32
                st["v16"] = v16

            def scores_head(j, h):
                st = state[j]
                qkT32 = st["qkT32"]
                qc, kc = QCOL[h], KCOL[h]
                sp = ps.tile([128, 384], F32, tag="sc", bufs=3,
                             name=f"s{h}_{j}")
                # S^T[tk, tq]; k0 = tk 0:128 (cols 0:256), k1 = tk 128:256
                # (cols 256:384, tq 128:256 only).  start=True clears
                # has_written for the whole 2KB psum bank, so each region's
                # start + accumulations stay contiguous.
                nc.tensor.matmul(
                    sp[:, 0:128],
                    qkT32[0:32, kc:kc + 128], qkT32[0:32, qc:qc + 128],
                    start=True, stop=False, tile_position=(0, 0),
                    skip_group_check=True)
                nc.tensor.matmul(
                    sp[:, 0:128], tri16[:], ident16[:],
                    start=False, stop=True, tile_position=(0, 0),
                    skip_group_check=True)
                nc.tensor.matmul(
                    sp[:, 128:256],
                    qkT32[0:32, kc:kc + 128],
                    qkT32[0:32, qc + 128:qc + 256],
                    start=True, stop=True, tile_position=(0, 0),
                    skip_group_check=True)
                nc.tensor.matmul(
                    sp[:, 256:384],
                    qkT32[0:32, kc + 128:kc + 256],
                    qkT32[0:32, qc + 128:qc + 256],
                    start=True, stop=False, tile_position=(0, 0),
                    skip_group_check=True)
                nc.tensor.matmul(
                    sp[:, 256:384], tri16[:], ident16[:],
                    start=False, stop=True, tile_position=(0, 0),
                    skip_group_check=True)
                at = sb.tile([128, 384], F16, tag=f"at{h}", name=f"at{h}_{j}")
                nc.scalar.activation(at[:], sp[:], Exp, scale=float(SCALE))
                st[f"at{h}"] = at

            def av_head(j, h):
                st = state[j]
                am = st[f"at{h}"]
                v16 = st["v16"]
                sums_ps, av_ps = st["sums_ps"], st["av_ps"]
                pb, cc = 32 * (h % 4), 256 * (h // 4)
                k0 = am[:, 0:256]
                k1 = am[:, 256:384]
                # strictly k0-then-k1; heads sharing partitions (h, h+4)
                # are issued far apart by construction
                nc.tensor.matmul(
                    sums_ps[pb:pb + 32, cc:cc + 256], ones16[:], k0,
                    start=True, stop=False, tile_position=(0, pb),
                    skip_group_check=True)
                nc.tensor.matmul(
                    sums_ps[pb:pb + 32, cc + 128:cc + 256], ones16[:], k1,
                    start=False, stop=True, tile_position=(0, pb),
                    skip_group_check=True)
                nc.tensor.matmul(
                    av_ps[pb:pb + 32, cc:cc + 256],
                    v16[:, 0, h * 32:h * 32 + 32], k0,
                    start=True, stop=False, tile_position=(0, pb),
                    skip_group_check=True)
                nc.tensor.matmul(
                    av_ps[pb:pb + 32, cc + 128:cc + 256],
                    v16[:, 1, h * 32:h * 32 + 32], k1,
                    start=False, stop=True, tile_position=(0, pb),
                    skip_group_check=True)

            def attn_a(j):
                st = state[j]
                st["sums_ps"] = ps.tile([128, 512], F32, tag="sm",
                                        name=f"sm{j}")
                st["av_ps"] = ps.tile([128, 512], F32, tag="av", name=f"av{j}")
                scores_head(j, 0)
                scores_head(j, 1)
                scores_head(j, 2)

            def attn_b(j):
                av_head(j, 0)
                scores_head(j, 3)
                av_head(j, 1)
                scores_head(j, 4)
                av_head(j, 2)
                scores_head(j, 5)

            def attn_c(j):
                st = state[j]
                sums_ps, av_ps = st["sums_ps"], st["av_ps"]
                av_head(j, 3)
                av_head(j, 4)
                av_head(j, 5)
                recip = sb.tile([128, 512], F32, tag="rc", name=f"rc{j}")
                nc.vector.reciprocal(recip[:, 0:256], sums_ps[:, 0:256])
                nc.vector.reciprocal(recip[0:64, 256:512],
                                     sums_ps[0:64, 256:512])
                yT0 = sb.tile([128, 256], F16, tag="yT0", name=f"yT0_{j}")
                yT1 = sb.tile([64, 256], F16, tag="yT1", name=f"yT1_{j}")
                nc.vector.tensor_mul(yT0[:], av_ps[:, 0:256], recip[:, 0:256])
                nc.vector.tensor_mul(yT1[:], av_ps[0:64, 256:512],
                                     recip[0:64, 256:512])
                st["yT0"] = yT0
                st["yT1"] = yT1

            def outproj(k):
                st = state.pop(k)
                yT0, yT1 = st["yT0"], st["yT1"]
                o_ps = ps.tile([128, 384], F32, tag="o", name=f"o{k}")
                for u in range(2):
                    nc.tensor.matmul(
                        o_ps[:, u * 192:u * 192 + 192],
                        yT0[:, u * 128:u * 128 + 128], woT[0][:],
                        start=True, stop=False)
                    nc.tensor.matmul(
                        o_ps[:, u * 192:u * 192 + 192],
                        yT1[:, u * 128:u * 128 + 128], woT[1][:],
                        start=False, stop=True)
                o_sb = sb.tile([128, 2, 192], F32, tag="ob", name=f"ob{k}")
                nc.scalar.copy(o_sb[:], o_ps[:])
                nc.sync.dma_start(o_v[k], o_sb[:])

            rep_cm = tc.For_i(0, repeat) if repeat else contextlib.nullcontext()
            with rep_cm:
                for i in range(bpc + 2):
                    j, k = i - 1, i - 2
                    if i == 0:
                        load(0)
                        trans(0)
                    if 0 <= j < bpc:
                        attn_a(j)
                    if i + 1 < bpc:
                        load(i + 1)
                    if 0 <= j < bpc:
                        attn_b(j)
                    if i < bpc:
                        front_qkv(i)
                    if i + 1 < bpc:
                        trans(i + 1)
                    if 0 <= j < bpc:
                        attn_c(j)
                    if 0 <= k < bpc:
                        outproj(k)

    nc.compile()
    return nc


def _get_nc():
    if "nc" not in _CACHE:
        _CACHE["nc"] = _build()
    return _CACHE["nc"]


def kernel(x: np.ndarray, w_qkv: np.ndarray, w_out: np.ndarray) -> np.ndarray:
    from concourse.bass_utils import run_bass_kernel_spmd

    nc = _get_nc()
    x = np.ascontiguousarray(np.asarray(x, dtype=np.float32))
    w_qkv = np.ascontiguousarray(np.asarray(w_qkv, dtype=np.float32))
    w_out = np.ascontiguousarray(np.asarray(w_out, dtype=np.float32))
    in_maps = [
        {"x": x[i * BPC:(i + 1) * BPC], "w_qkv": w_qkv, "w_out": w_out}
        for i in range(NCORES)
    ]
    res = run_bass_kernel_spmd(nc, in_maps, core_ids=list(range(NCORES)))
    out = np.concatenate([r["out"] for r in res.results], axis=0)
    return out.astype(np.float32)
